# revision 1
# baseline (speedup 1.0000x reference)
import math
import sys

sys.path.insert(0, "/opt/trn_rl_repo")

import numpy as np

# Problem constants (hardcoded per spec)
NQ = 12
SEQ = 16
DD = 3
DIM = 1 << NQ
B_FULL = 2048
N_CORES = 8
B_LOC = B_FULL // N_CORES  # 256 samples per core
P = 128                    # partition tile (samples per tile)

_CACHE = {}


def _pbcast(bass, ap, prt):
    """Broadcast a DRAM tensor (no partition dim) across prt partitions."""
    return bass.AP(tensor=ap.tensor, offset=ap.offset, ap=[[0, prt]] + [list(d) for d in ap.ap])


def _mkv(bass, t, off, dims):
    """Manual strided view of a [prt, dim] tile (element offsets/steps)."""
    a = t[:, :]
    return bass.AP(
        tensor=a.tensor,
        offset=a.offset + off,
        ap=[list(a.ap[0])] + [list(d) for d in dims],
    )


def _contig_like(bass, scratch, off, ref_ap):
    """Contiguous view of `scratch` (from element `off`) shaped like ref_ap's free dims."""
    dims = [list(d) for d in ref_ap.ap[1:]]
    cont = []
    stride = 1
    for d in reversed(dims):
        cont.insert(0, [stride, d[1]])
        stride *= d[1]
    return _mkv(bass, scratch, off, cont), stride


def _gate_views(bass, Apl, Bpl, j, pending, nq):
    """List of (A0, A1, B0, B1) view-slices for the RY stage of a gate on wire j.

    A is read through the not-yet-materialized CNOT permutation `pending`
    (None | 'chain' = C(j, j+1 mod nq) | 'g1' = C(0,1) then C(1,2), j==1);
    B views are in the true basis, so the gate's full-state write
    materializes the permutation. Slicing by the true target-bit value
    keeps every view 2D/3D (walrus STT limit) with positive steps only.
    """
    dim = 1 << nq
    s = 1 << (nq - 1 - j)
    if pending is None:
        nb = dim // (2 * s)
        d0 = [[2 * s, nb], [1, s]]
        return [(
            _mkv(bass, Apl, 0, d0), _mkv(bass, Apl, s, d0),
            _mkv(bass, Bpl, 0, d0), _mkv(bass, Bpl, s, d0),
        )]
    if pending == "chain":
        if j < nq - 1:
            st = s // 2
            nb = dim // (2 * s)
            if st == 1:
                # tau-split views here would be stride-16B singles (~16 ns/elem
                # SBUF fetch penalty). The split collapses to 2 free dims, so
                # merge both taus into one op; A1 takes a reversed inner pair.
                d = [[2 * s, nb], [1, 2]]
                return [(
                    _mkv(bass, Apl, 0, d),
                    _mkv(bass, Apl, s + st, [[2 * s, nb], [-1, 2]]),
                    _mkv(bass, Bpl, 0, d),
                    _mkv(bass, Bpl, s, d),
                )]
            d = [[2 * s, nb], [1, st]]
            out = []
            for tau in (0, 1):
                out.append((
                    _mkv(bass, Apl, tau * st, d),
                    _mkv(bass, Apl, s + (1 - tau) * st, d),
                    _mkv(bass, Bpl, tau * st, d),
                    _mkv(bass, Bpl, s + tau * st, d),
                ))
            return out
        else:  # wrap: ctrl = nq-1, tgt = 0
            s0 = dim // 2
            d = [[2, s0 // 2]]
            out = []
            for tau in (0, 1):
                out.append((
                    _mkv(bass, Apl, tau * s0, d),
                    _mkv(bass, Apl, (1 - tau) * s0 + 1, d),
                    _mkv(bass, Bpl, tau * s0, d),
                    _mkv(bass, Bpl, tau * s0 + 1, d),
                ))
            return out
    assert pending == "g1" and j == 1
    s0, s1, s2 = dim // 2, dim // 4, dim // 8
    out = []
    for tau in (0, 1):
        out.append((
            _mkv(bass, Apl, tau * s2, [[s0 + s1, 2], [1, s2]]),
            _mkv(bass, Apl, s1 + (1 - tau) * s2, [[s0 - s1, 2], [1, s2]]),
            _mkv(bass, Bpl, tau * s2, [[s0, 2], [1, s2]]),
            _mkv(bass, Bpl, s1 + tau * s2, [[s0, 2], [1, s2]]),
        ))
    return out


def _ring_sign_wires(nq):
    """Per group g: wire set W' with parity_{W'}(s) == parity_{W_g}(ring(s))."""
    dim = 1 << nq
    s = np.arange(dim)
    for q in range(nq):
        c, t = (q, q + 1) if q < nq - 1 else (nq - 1, 0)
        s = s ^ (((s >> (nq - 1 - c)) & 1) << (nq - 1 - t))
    n3 = nq // 3
    out = []
    for g in range(3):
        par = np.zeros(dim, dtype=np.int64)
        for w in range(g * n3, (g + 1) * n3):
            par ^= (s >> (nq - 1 - w)) & 1
        sign = 1 - 2 * par
        wires = [w for w in range(nq) if sign[1 << (nq - 1 - w)] == -1]
        chk = np.zeros(dim, dtype=np.int64)
        for w in wires:
            chk ^= (np.arange(dim) >> (nq - 1 - w)) & 1
        assert np.array_equal(1 - 2 * chk, sign), "ring sign factorization failed"
        out.append(wires)
    return out


def build_program(nq=NQ, seq=SEQ, b_loc=B_LOC, n_cores=N_CORES, repeat=1,
                  route_b1r=None, route_b1i="ag", route_b0i=None, apply_gp=1):
    """Build and compile the per-core SPMD Bass program.

    The per-step operator is regrouped as (⊗_j RZ_j)·(⊗_j RY_j): the 12
    deferred-scale RY Givens passes stay per-wire (stt), while the 12 RZ
    phase gates collapse into ONE tensor-product diagonal built by
    doubling and applied as a complex elementwise multiply. The per-step
    cos-product renorm is folded into the diagonal's seed. RY FMA units
    can be routed off the (bottleneck) DVE: 'ad' = ACT mult + DVE add,
    'ag' = ACT mult + GPSIMD add.
    """
    key = (nq, seq, b_loc, n_cores, repeat, route_b1r, route_b1i, route_b0i, apply_gp)
    if key in _CACHE:
        return _CACHE[key]

    import concourse.bass as bass
    import concourse.bacc as bacc
    import concourse.tile as tile
    from concourse import mybir

    FP = mybir.dt.float32
    FPH = mybir.dt.float16
    AF = mybir.ActivationFunctionType
    ALU = mybir.AluOpType
    AX = mybir.AxisListType.X

    dim = 1 << nq
    prt = min(P, b_loc)
    n_tiles = max(1, b_loc // prt)
    ng = seq * nq

    nc = bacc.Bacc("TRN2", target_bir_lowering=False, debug=False, num_devices=n_cores)
    x_ext = nc.dram_tensor("x", [b_loc, seq, DD], FP, kind="ExternalInput").ap()
    w_ext = nc.dram_tensor("w", [seq, nq, 2 * DD], FP, kind="ExternalInput").ap()
    b_ext = nc.dram_tensor("b", [seq, nq, 2], FP, kind="ExternalInput").ap()
    y_ext = nc.dram_tensor("y", [b_loc, 3], FP, kind="ExternalOutput").ap()

    with tile.TileContext(nc) as tc:
        with (
            tc.tile_pool(name="state", bufs=1) as st,
            tc.tile_pool(name="scal", bufs=1) as sc,
            tc.tile_pool(name="tmp", bufs=2) as tp,
            tc.tile_pool(name="scr", bufs=1) as scr,
        ):
            stt = nc.vector.scalar_tensor_tensor
            tt = nc.vector.tensor_tensor
            gtt = nc.gpsimd.tensor_tensor

            # shared across tiles: diagonal (fp16; built mostly in fp32) + scratch
            Dr = scr.tile([prt, dim], FPH, tag="Dr")
            Di = scr.tile([prt, dim], FPH, tag="Di")
            Tb = scr.tile([prt, dim // 4], FP, tag="Tb")  # build swap scratch
            # replicated weights (same for both tiles)
            wrep = scr.tile([prt, seq, nq, 2 * DD], FP, tag="wrep")
            nc.sync.dma_start(out=wrep, in_=_pbcast(bass, w_ext, prt))
            brep = scr.tile([prt, seq, nq, 2], FP, tag="brep")
            nc.sync.dma_start(out=brep, in_=_pbcast(bass, b_ext, prt))
            nc.vector.tensor_scalar_mul(wrep, wrep, 0.5)
            nc.vector.tensor_scalar_mul(brep, brep, 0.5)

            for tidx in range(n_tiles):
                # ---------- inputs ----------
                xt = sc.tile([prt, seq, DD], FP, tag=f"xt{tidx}")
                nc.sync.dma_start(out=xt, in_=x_ext[tidx * prt:(tidx + 1) * prt])

                # ---------- angles: h = 0.5*(x . W) + 0.5*bias ----------
                xrep = sc.tile([prt, seq, nq, DD], FP, tag=f"xrep{tidx}")
                for j in range(nq):
                    nc.vector.tensor_copy(xrep[:, :, j, :], xt)
                h = []
                for half in range(2):
                    prod = tp.tile([prt, seq, nq, DD], FP, tag="prod")
                    tt(prod, xrep, wrep[:, :, :, half * DD:(half + 1) * DD], ALU.mult)
                    hv = sc.tile([prt, seq, nq], FP, tag=f"h{half}_{tidx}")
                    nc.vector.tensor_reduce(hv, prod, axis=AX, op=ALU.add)
                    tt(hv, hv, brep[:, :, :, half], ALU.add)
                    h.append(hv)

                # ---------- per-gate scalars ----------
                # t = tan(h1), w = -tan(h2); per-step renorm g = prod_j cos(h1)cos(h2)
                MAGIC = 1.5 * (2.0 ** 23)  # fp32 round-to-nearest-int trick
                TWO_PI = 2.0 * math.pi

                def trig(hv, tag):
                    # sin/cos of unbounded h via range reduction to [-pi, pi]
                    cv = sc.tile([prt, ng], FP, tag=f"c{tag}")
                    sv = tp.tile([prt, ng], FP, tag="sv")
                    hf = hv.rearrange("p a b -> p (a b)")
                    m = tp.tile([prt, ng], FP, tag="m")
                    nc.vector.tensor_scalar(m, hf, 1.0 / TWO_PI, None, ALU.mult)
                    k = tp.tile([prt, ng], FP, tag="k")
                    r = tp.tile([prt, ng], FP, tag="r")
                    # sin: r = m - round(m); x = 2*pi*r
                    nc.vector.tensor_scalar(k, m, MAGIC, MAGIC, ALU.add, ALU.subtract)
                    tt(r, m, k, ALU.subtract)
                    nc.vector.tensor_scalar(r, r, TWO_PI, None, ALU.mult)
                    nc.scalar.activation(sv, r, AF.Sin)
                    # cos: shift phase by +pi/2 (m + 0.25 turns)
                    mc = tp.tile([prt, ng], FP, tag="mc")
                    nc.vector.tensor_scalar(mc, m, 0.25, None, ALU.add)
                    nc.vector.tensor_scalar(k, mc, MAGIC, MAGIC, ALU.add, ALU.subtract)
                    tt(r, mc, k, ALU.subtract)
                    nc.vector.tensor_scalar(r, r, TWO_PI, None, ALU.mult)
                    nc.scalar.activation(cv, r, AF.Sin)
                    rcv = tp.tile([prt, ng], FP, tag="rcv")
                    nc.vector.reciprocal(rcv, cv)
                    dv = sc.tile([prt, ng], FP, tag=f"d{tag}")
                    tt(dv, sv, rcv, ALU.mult)
                    return cv, dv

                c1, t_ = trig(h[0], f"1_{tidx}")   # t_ = tan(h1)
                c2, wm = trig(h[1], f"2_{tidx}")   # wm = tan(h2) = -w
                tm = sc.tile([prt, ng], FP, tag=f"tm{tidx}")
                nc.vector.tensor_scalar_mul(tm, t_, -1.0)
                w_ = sc.tile([prt, ng], FP, tag=f"w{tidx}")
                nc.vector.tensor_scalar_mul(w_, wm, -1.0)
                gg = sc.tile([prt, ng], FP, tag=f"gg{tidx}")
                tt(gg, c1, c2, ALU.mult)
                # per-step product over the nq gates (pairwise tree; no mult-reduce)
                cur = gg.rearrange("p (a b) -> p a b", b=nq)
                n = nq
                lvl = 0
                while n > 1:
                    hn = n // 2
                    nxt = sc.tile([prt, seq, hn], FP, tag=f"gl{lvl}_{tidx}")
                    tt(nxt, cur[:, :, :hn], cur[:, :, hn:2 * hn], ALU.mult)
                    if n % 2:
                        tt(nxt[:, :, 0:1], nxt[:, :, 0:1], cur[:, :, n - 1:n], ALU.mult)
                    cur, n, lvl = nxt, hn, lvl + 1
                gcol = cur.rearrange("p a b -> p (a b)")

                # per-tile scratch: routed ACT mults, fp32 diag-build levels,
                # and the complex-apply cross terms (all serialized in time)
                S = scr.tile([prt, dim // 2], FP, tag=f"S{tidx}")

                # ---------- batched diag-build prefix ----------
                # Seeds + the 3 smallest doubling levels for ALL 15 steps in
                # one series (tiny per-step DVE ops are latency-bound at
                # ~0.7us each; batching via stride-0 broadcast W views turns
                # 19 ops/step into ~32 ops/tile). Dsm[i] = 16-entry diagonal
                # prefix (wires 8..11) of step i+1, seeded with g_{i+1}.
                nst = seq - 1
                DsmR = scr.tile([prt, nst * 16], FP, tag=f"DsmR{tidx}")
                DsmI = scr.tile([prt, nst * 16], FP, tag=f"DsmI{tidx}")
                gv = gcol[:, 1:seq]
                nc.vector.tensor_copy(_mkv(bass, DsmR, 0, [[16, nst]]), gv)
                nc.vector.tensor_copy(_mkv(bass, DsmR, 1, [[16, nst]]), gv)
                tt(_mkv(bass, DsmI, 0, [[16, nst]]),
                   _mkv(bass, w_, nq + (nq - 1), [[nq, nst]]), gv, ALU.mult)
                tt(_mkv(bass, DsmI, 1, [[16, nst]]),
                   _mkv(bass, wm, nq + (nq - 1), [[nq, nst]]), gv, ALU.mult)
                for jb in (nq - 2, nq - 3, nq - 4):
                    m = 1 << (nq - 1 - jb)
                    wj = _mkv(bass, w_, nq + jb, [[nq, nst], [0, m]])
                    wmj = _mkv(bass, wm, nq + jb, [[nq, nst], [0, m]])
                    Zr = _mkv(bass, DsmR, 0, [[16, nst], [1, m]])
                    Zi = _mkv(bass, DsmI, 0, [[16, nst], [1, m]])
                    Hr = _mkv(bass, DsmR, m, [[16, nst], [1, m]])
                    Hi = _mkv(bass, DsmI, m, [[16, nst], [1, m]])
                    tl = [_mkv(bass, Tb, q * nst * m, [[m, nst], [1, m]])
                          for q in range(4)]
                    # bit1 blocks: Hr = Zi*w + Zr ; Hi = Zr*wm + Zi
                    tt(tl[0], Zi, wj, ALU.mult)
                    tt(Hr, tl[0], Zr, ALU.add)
                    tt(tl[1], Zr, wmj, ALU.mult)
                    tt(Hi, tl[1], Zi, ALU.add)
                    # bit0 blocks in place: Zi += Zr*w ; Zr += Zi_old*wm
                    tt(tl[2], Zr, wj, ALU.mult)
                    tt(tl[3], Zi, wmj, ALU.mult)
                    tt(Zi, tl[2], Zi, ALU.add)
                    tt(Zr, tl[3], Zr, ALU.add)

                for _rep in range(repeat):
                    # ---------- state init: step-0 product state ----------
                    # After step 0, psi = prod_j (e_j c_j, conj(e_j) s_j); in the
                    # deferred-scale basis v'_j = (1 + i*w_j, t_j*(1 - i*w_j)),
                    # seeded with g_0 so no separate renorm pass is needed.
                    Ar = st.tile([prt, dim], FP, tag=f"Ar{tidx}")
                    Ai = st.tile([prt, dim], FP, tag=f"Ai{tidx}")
                    Br = st.tile([prt, dim], FP, tag=f"Br{tidx}")
                    Bi = st.tile([prt, dim], FP, tag=f"Bi{tidx}")
                    g0 = gcol[:, 0:1]
                    nc.vector.tensor_copy(Br[:, 0:1], g0)
                    tt(Br[:, 1:2], t_[:, 0:1], g0, ALU.mult)
                    tt(Bi[:, 0:1], w_[:, 0:1], g0, ALU.mult)
                    tt(Bi[:, 1:2], t_[:, 0:1], wm[:, 0:1], ALU.mult)
                    tt(Bi[:, 1:2], Bi[:, 1:2], g0, ALU.mult)
                    Xr, Xi, Yr, Yi = Br, Bi, Ar, Ai
                    for jw in range(1, nq):
                        m = 1 << jw
                        wcj = w_[:, jw:jw + 1]
                        wmj = wm[:, jw:jw + 1]
                        tj = t_[:, jw:jw + 1]
                        Yvr = Yr[:, :2 * m].rearrange("p (m two) -> p m two", two=2)
                        Yvi = Yi[:, :2 * m].rearrange("p (m two) -> p m two", two=2)
                        stt(Yvr[:, :, 0], Xi[:, :m], wmj, Xr[:, :m], ALU.mult, ALU.add)
                        stt(Yvi[:, :, 0], Xr[:, :m], wcj, Xi[:, :m], ALU.mult, ALU.add)
                        stt(Xr[:, m:2 * m], Xi[:, :m], wcj, Xr[:, :m], ALU.mult, ALU.add)
                        stt(Xi[:, m:2 * m], Xr[:, :m], wmj, Xi[:, :m], ALU.mult, ALU.add)
                        nc.vector.tensor_scalar(Yvr[:, :, 1], Xr[:, m:2 * m], tj, None, ALU.mult)
                        nc.vector.tensor_scalar(Yvi[:, :, 1], Xi[:, m:2 * m], tj, None, ALU.mult)
                        Xr, Xi, Yr, Yi = Yr, Yi, Xr, Xi
                    assert Xr is Ar
                    CUr, CUi, OTr, OTi = Ar, Ai, Br, Bi  # cur = A after init

                    # ---------- evolution ----------
                    # Step i's CNOT ring is folded into step i+1's gate reads:
                    # schedule [C01 C12] G1 [C23] G2 ... [C(10,11)] G10 [C(11,0)] G11 G0.
                    # The last step's ring is folded into the observable signs.
                    for i in range(1, seq):
                        order = [(1, "g1")] + [(j, "chain") for j in range(2, nq)] \
                                + [(0, None)]
                        # --- RY Givens per wire (deferred cos scale) ---
                        sc_g = nc.named_scope(f"gates")
                        sc_g.__enter__()
                        for (j, pend) in order:
                            k = i * nq + j
                            slr = _gate_views(bass, CUr, OTr, j, pend, nq)
                            sli = _gate_views(bass, CUi, OTi, j, pend, nq)
                            tc_ = t_[:, k:k + 1]
                            tmc = tm[:, k:k + 1]
                            soff = 0

                            def unit(dst, src_m, src_a, scal, route):
                                # dst = src_m * scal + src_a, routed per `route`
                                nonlocal soff
                                if route is None:
                                    stt(dst, src_m, scal, src_a, ALU.mult, ALU.add)
                                    return
                                u, sz = _contig_like(bass, S, soff, src_m)
                                soff += sz
                                nc.scalar.activation(u, src_m, AF.Copy, bias=0.0, scale=scal)
                                if route == "ad":
                                    tt(dst, u, src_a, ALU.add)
                                else:  # 'ag'
                                    gtt(dst, u, src_a, ALU.add)

                            # wire nq-2's merged views use a reversed inner
                            # pair on A1 — keep it off ACT/GPSIMD
                            rb0i, rb1r, rb1i = route_b0i, route_b1r, route_b1i
                            if pend == "chain" and j == nq - 2:
                                rb0i = rb1r = rb1i = None
                            for si in range(len(slr)):
                                A0r, A1r, B0r, B1r = slr[si]
                                A0i, A1i, B0i, B1i = sli[si]
                                # B0 = A0 - t*A1 ; B1 = A1 + t*A0
                                stt(B0r, A1r, tmc, A0r, ALU.mult, ALU.add)
                                unit(B0i, A1i, A0i, tmc, rb0i)
                                unit(B1r, A0r, A1r, tc_, rb1r)
                                unit(B1i, A0i, A1i, tc_, rb1i)
                            CUr, CUi, OTr, OTi = OTr, OTi, CUr, CUi

                        sc_g.__exit__(None, None, None)
                        # --- batched RZ: D = g_i * prod_j (1 -+ i w_j) ---
                        # Doubling from the LSB wire. Levels up to 1024 entries
                        # live in fp32 in S (Zr = S[0:1024], Zi = S[1024:2048]);
                        # the last two levels write the fp16 D, so the fp16
                        # rounding is paid only twice.
                        sc_b = nc.named_scope("build")
                        sc_b.__enter__()
                        hq = dim // 4  # 1024
                        row = (i - 1) * 16
                        nc.vector.tensor_copy(S[:, 0:16], _mkv(bass, DsmR, row, [[1, 16]]))
                        nc.vector.tensor_copy(S[:, hq:hq + 16], _mkv(bass, DsmI, row, [[1, 16]]))
                        m = 16
                        for j in range(nq - 5, -1, -1):
                            k = i * nq + j
                            wc = w_[:, k:k + 1]
                            wmc = wm[:, k:k + 1]
                            if m <= hq // 2:
                                # in-S fp32 level: bit1 out-of-place, bit0 via Tb
                                stt(S[:, m:2 * m], S[:, hq:hq + m], wc, S[:, 0:m], ALU.mult, ALU.add)
                                stt(S[:, hq + m:hq + 2 * m], S[:, 0:m], wmc, S[:, hq:hq + m], ALU.mult, ALU.add)
                                stt(Tb[:, 0:m], S[:, 0:m], wc, S[:, hq:hq + m], ALU.mult, ALU.add)
                                stt(S[:, 0:m], S[:, hq:hq + m], wmc, S[:, 0:m], ALU.mult, ALU.add)
                                nc.vector.tensor_copy(S[:, hq:hq + m], Tb[:, 0:m])
                            elif m == hq:
                                # S(fp32) -> D(fp16), fully out-of-place
                                stt(Dr[:, m:2 * m], S[:, hq:hq + m], wc, S[:, 0:m], ALU.mult, ALU.add)
                                stt(Di[:, m:2 * m], S[:, 0:m], wmc, S[:, hq:hq + m], ALU.mult, ALU.add)
                                stt(Dr[:, 0:m], S[:, hq:hq + m], wmc, S[:, 0:m], ALU.mult, ALU.add)
                                stt(Di[:, 0:m], S[:, 0:m], wc, S[:, hq:hq + m], ALU.mult, ALU.add)
                            else:
                                # final in-D level; bit0 in place via Tb chunks
                                stt(Dr[:, m:2 * m], Di[:, 0:m], wc, Dr[:, 0:m], ALU.mult, ALU.add)
                                stt(Di[:, m:2 * m], Dr[:, 0:m], wmc, Di[:, 0:m], ALU.mult, ALU.add)
                                for c0 in range(0, m, hq):
                                    cs = slice(c0, c0 + hq)
                                    stt(Tb[:, 0:hq], Dr[:, cs], wc, Di[:, cs], ALU.mult, ALU.add)
                                    stt(Dr[:, cs], Di[:, cs], wmc, Dr[:, cs], ALU.mult, ALU.add)
                                    nc.vector.tensor_copy(Di[:, cs], Tb[:, 0:hq])
                            m *= 2
                        assert m == dim
                        sc_b.__exit__(None, None, None)
                        sc_a = nc.named_scope("apply")
                        sc_a.__enter__()
                        # --- apply: OT = CU * D (complex), in halves with S scratch ---
                        mul1 = gtt if apply_gp >= 1 else tt
                        mul2 = gtt if apply_gp >= 2 else tt
                        hd = dim // 2
                        SA = S[:, 0:hd]
                        for h0 in (0, hd):
                            hs = slice(h0, h0 + hd)
                            mul1(SA, CUi[:, hs], Di[:, hs], ALU.mult)
                            tt(OTr[:, hs], CUr[:, hs], Dr[:, hs], ALU.mult)
                            tt(OTr[:, hs], OTr[:, hs], SA, ALU.subtract)
                            mul2(SA, CUi[:, hs], Dr[:, hs], ALU.mult)
                            tt(OTi[:, hs], CUr[:, hs], Di[:, hs], ALU.mult)
                            tt(OTi[:, hs], OTi[:, hs], SA, ALU.add)
                        sc_a.__exit__(None, None, None)
                        CUr, CUi, OTr, OTi = OTr, OTi, CUr, CUi

                    # ---------- observables ----------
                    # p = |psi|^2, then 3 signed halving trees
                    Pr, Pi = OTr, OTi
                    nc.scalar.activation(Pr, CUr, AF.Square)
                    nc.scalar.activation(Pi, CUi, AF.Square)
                    tt(Pr, Pr, Pi, ALU.add)
                    out_t = tp.tile([prt, 3], FP, tag="out")
                    ring_wires = _ring_sign_wires(nq)
                    for grp in range(3):
                        wires = ring_wires[grp]
                        cur = Pr[:, :dim]
                        cur_n = dim
                        off = 0
                        for wi in wires:
                            s = 1 << (nq - 1 - wi)
                            a = cur_n // (2 * s)
                            v = cur.rearrange("p (a two s) -> p a two s", two=2, s=s, a=a)
                            nxt_n = cur_n // 2
                            nxt = Pi[:, off:off + nxt_n]
                            off += nxt_n
                            nv = nxt.rearrange("p (a s) -> p a s", a=a, s=s)
                            tt(nv, v[:, :, 0, :], v[:, :, 1, :], ALU.subtract)
                            cur = nxt
                            cur_n = nxt_n
                        ex = tp.tile([prt, 1], FP, tag="ex")
                        nc.vector.tensor_reduce(ex, cur, axis=AX, op=ALU.add)
                        # out = (exp + 1) / 2
                        nc.vector.tensor_scalar(
                            out_t[:, grp:grp + 1], ex, 0.5, 0.5, ALU.mult, ALU.add
                        )
                    nc.sync.dma_start(
                        out=y_ext[tidx * prt:(tidx + 1) * prt], in_=out_t
                    )

    nc.compile()
    _CACHE[key] = nc
    return nc


def kernel(x, weights, bias):
    from concourse.bass_utils import run_bass_kernel_spmd

    nc = build_program()
    in_maps = [
        {
            "x": np.ascontiguousarray(x[i * B_LOC:(i + 1) * B_LOC], dtype=np.float32),
            "w": np.ascontiguousarray(weights, dtype=np.float32),
            "b": np.ascontiguousarray(bias, dtype=np.float32),
        }
        for i in range(N_CORES)
    ]
    res = run_bass_kernel_spmd(nc, in_maps, list(range(N_CORES)))
    return np.concatenate([res.results[i]["y"] for i in range(N_CORES)], axis=0)

